# revision 34
# baseline (speedup 1.0000x reference)
"""Trainium2 Bass kernel for nn_Loss5 (topk_masking).

reference:
    s_topk = top_k(x, 6)[0][:, 5]            # 6th largest per row, [B]
    s_y    = x[arange(B), y]                 # label score, [B]
    out    = mean(relu(1 + s_topk[None,:] - s_y[:,None]))   # over [B,B]

Strategy (8 cores, data-parallel over rows):
  - each core gets 512 rows of x ([512, 50257] f32, contiguous slice)
  - stage 1: exact per-row top-8 via the DVE Max8 instruction
    (nc.vector.max) over column chunks, then a final Max8 over the
    per-chunk candidates; t_j = 6th value (local rows only).
  - the loss is sharded over j (columns of the [B_i, B_j] clamp
    matrix): core c computes P_c = sum_{j in rows_c} sum_i
    relu((1 - s_i) + t_j).  This needs ALL s_i but only LOCAL t_j, so
    the collective moves to the *front* where it is hidden behind the
    stage-1 DMA stream: gather s_y by indirect DMA (~us), compute
    (1 - s_y), AllGather it ([B] floats, tiny), broadcast to a
    [128, B] SBUF tile.  Per row-group, one ACT pass
    relu(sneg_i + t_j) with accum_out runs as soon as that group's
    top-k lands, overlapped with the remaining DMA.
  - the last group streams in ~2.4k-col chunks ending in a graduated
    tail (1700..400) so the DVE Max8 drain after the final byte is
    ~2.5us instead of 8.8us; the final loss pass is split between ACT
    (relu+accum fused) and DVE (4x-mode relu + reduce) so both legs
    finish together ~6.6us after the last byte.
  - host: sum 8x[128,G+1] partials, divide by B^2.

Raw bass blocks (not Tile): the toolchain's DMA pseudo-instructions
support only ONE attached sync wait, and Tile's wait emission is not
transitively minimal (slot reuse emits RAW + WAW waits on one DMA).
With explicit semaphores the WAW is implied transitively and every DMA
carries at most one wait.
"""

import sys

import numpy as np

if "/opt/trn_rl_repo" not in sys.path:
    sys.path.insert(0, "/opt/trn_rl_repo")

import concourse.bass as bass
import concourse.mybir as mybir
from concourse.bass_utils import run_bass_kernel_spmd

B = 4096
V = 50257
NCORES = 8
RPC = B // NCORES          # rows per core = 512
G = RPC // 128             # row groups of 128 partitions = 4
K = 5                      # s_topk = (K+1)-th largest = top8[:, 5]

# column chunking for Max8 (input free size must be <= 16384).
# groups 0..G-2: 6 roughly equal chunks.  last group: small equal chunks
# (~2400: keeps the DVE caught up with the DMA stream, DVE(w)+sem <
# DMA(w)) ending in a graduated tail so the DVE drain after the last
# byte shrinks from 8.8us (one big chunk) to ~2us.
def _even_chunks(total, n):
    out, c0 = [], 0
    base, rem = total // n, total % n
    for j in range(n):
        w = base + (1 if j < rem else 0)
        out.append((c0, w))
        c0 += w
    return out

_CHUNKS_STD = _even_chunks(V, 6)            # widths 8377/8376, max 8377
# graduated drain tail: DVE backlog recursion
#   lag(c) = max(lag(c-1) - DMA(w_c), 0.9us) + DVE(w_c)
# converges to ~1.7us with this profile (vs 8.8us for a full-width
# last chunk), robust across the measured DMA-rate range.
_TAILS = [2100, 1800, 1500, 1250, 1050, 880, 740, 620, 520, 440]
_CHUNKS_LAST = _even_chunks(V - sum(_TAILS), 16)
for _w in _TAILS:
    _c0 = _CHUNKS_LAST[-1][0] + _CHUNKS_LAST[-1][1]
    _CHUNKS_LAST.append((_c0, _w))
assert _CHUNKS_LAST[-1][0] + _CHUNKS_LAST[-1][1] == V
_GROUP_CHUNKS = [_CHUNKS_STD] * (G - 1) + [_CHUNKS_LAST]
W0 = max(w for ch in _GROUP_CHUNKS for _, w in ch)   # slot width
NCHUNK_TOT = sum(len(ch) for ch in _GROUP_CHUNKS)    # 25
# per-group candidate offsets into cand (8 slots per chunk)
_CAND_OFF = []
_off = 0
for ch in _GROUP_CHUNKS:
    _CAND_OFF.append(_off)
    _off += 8 * len(ch)
CANDW = _off

NSLOT = 5                  # x-tile load slots

# the last group's loss pass is split between ACT (relu+accum fused,
# 0.833 ns/elem) and DVE (4x-mode relu 0.26 + 1x reduce 1.04 ns/elem)
# so both legs finish together on the tail.
SPLIT_ACT = 2377           # ACT columns; DVE gets B - SPLIT_ACT = 1719

_NC_CACHE = {}


def _build_nc(repeat: int = 1):
    f32 = mybir.dt.float32
    bf16 = mybir.dt.bfloat16
    i32 = mybir.dt.int32

    nc = bass.Bass()
    x = nc.declare_dram_parameter("x", [RPC, V], f32, isOutput=False)
    syoff = nc.declare_dram_parameter("syoff", [128, G], i32, isOutput=False)
    partial = nc.declare_dram_parameter("partial", [128, G + 1], f32, isOutput=True)

    sloc_d = nc.dram_tensor("sloc_cc_in", [RPC], bf16)
    sfull_d = nc.dram_tensor("sfull_cc_out", [B], bf16, addr_space="Shared")

    x_flat = x.ap().rearrange("a b -> (a b)")[:, None]

    from contextlib import ExitStack

    with ExitStack() as ctx:
        slots = ctx.enter_context(nc.sbuf_tensor("slots", [128, NSLOT * W0], f32))
        cand = ctx.enter_context(nc.sbuf_tensor("cand", [128, CANDW], f32))
        top8 = ctx.enter_context(nc.sbuf_tensor("top8", [128, G * 8], f32))
        tbias = ctx.enter_context(nc.sbuf_tensor("tbias", [128, G], f32))
        sy_sb = ctx.enter_context(nc.sbuf_tensor("sy", [128, G], f32))
        sneg_loc = ctx.enter_context(nc.sbuf_tensor("snegl", [128, G], bf16))
        acc_sb = ctx.enter_context(nc.sbuf_tensor("acc", [128, G + 1], f32))
        so_sb = ctx.enter_context(nc.sbuf_tensor("so", [128, G], i32))
        sneg = ctx.enter_context(nc.sbuf_tensor("sneg", [128, B], bf16))
        scratch = ctx.enter_context(nc.sbuf_tensor("scratch", [128, B], bf16))
        warm = ctx.enter_context(nc.sbuf_tensor("warm", [128, 8], f32))
        ld0 = ctx.enter_context(nc.semaphore("ld0"))
        ld1 = ctx.enter_context(nc.semaphore("ld1"))
        ld2 = ctx.enter_context(nc.semaphore("ld2"))
        ld3 = ctx.enter_context(nc.semaphore("ld3"))
        ld4 = ctx.enter_context(nc.semaphore("ld4"))
        mx = ctx.enter_context(nc.semaphore("mx"))
        so_s = ctx.enter_context(nc.semaphore("so_s"))
        gat = ctx.enter_context(nc.semaphore("gat"))
        bias_s = ctx.enter_context(nc.semaphore("bias_s"))
        tcp = ctx.enter_context(nc.semaphore("tcp"))
        sld = ctx.enter_context(nc.semaphore("sld"))
        cc = ctx.enter_context(nc.semaphore("cc"))
        tb_s = ctx.enter_context(nc.semaphore("tb_s"))
        act = ctx.enter_context(nc.semaphore("act"))
        outs = ctx.enter_context(nc.semaphore("outs"))
        fmx = ctx.enter_context(nc.semaphore("fmx"))
        warm_s = ctx.enter_context(nc.semaphore("warm_s"))
        block = ctx.enter_context(nc.Block())
        ld_sems = [ld0, ld1, ld2, ld3, ld4]

        @block.sync
        def _(sync):
            k = 0
            for rep in range(repeat):
                for g in range(G):
                    for j, (c0, w) in enumerate(_GROUP_CHUNKS[g]):
                        if k >= NSLOT:
                            # reader of this slot's previous contents done
                            # (transitively implies the previous load completed)
                            sync.wait_ge(mx, k - NSLOT + 1)
                        s = (k % NSLOT) * W0
                        cs = (c0 + rep * 1237) % (V - w) if rep else c0
                        sync.dma_start(
                            out=slots[:, s : s + w],
                            in_=x[g * 128 : (g + 1) * 128, cs : cs + w],
                        ).then_inc(ld_sems[k % NSLOT], 16)
                        k += 1
            sync.wait_ge(act, G + 1)
            sync.dma_start(out=partial.ap(), in_=acc_sb[:]).then_inc(outs, 16)
            sync.wait_ge(outs, 16)

        @block.vector
        def _(vector):
            k = 0
            for rep in range(repeat):
                for g in range(G):
                    chunks = _GROUP_CHUNKS[g]
                    co = _CAND_OFF[g]
                    for j, (c0, w) in enumerate(chunks):
                        s = (k % NSLOT) * W0
                        vector.wait_ge(ld_sems[k % NSLOT], 16 * (k // NSLOT + 1))
                        nc.vector.max(
                            cand[:, co + 8 * j : co + 8 * j + 8],
                            slots[:, s : s + w],
                        ).then_inc(mx, 1)
                        k += 1
                    vector.wait_ge(mx, k)
                    nc.vector.max(
                        top8[:, 8 * g : 8 * g + 8],
                        cand[:, co : co + 8 * len(chunks)],
                    ).then_inc(fmx, 1)
                    vector.wait_ge(fmx, rep * G + g + 1)
                    nc.vector.tensor_copy(
                        tbias[:, g : g + 1], top8[:, 8 * g + K : 8 * g + K + 1]
                    ).then_inc(tcp, 1)
                    if g == G - 1 and rep == repeat - 1:
                        # DVE leg of the last group's loss pass:
                        # relu(sneg + t) elementwise (4x mode on bf16), then
                        # a 1x reduce-add into acc[:, G].  ACT covers the
                        # first SPLIT_ACT columns concurrently.
                        vector.wait_ge(tb_s, 16)
                        vector.wait_ge(act, G - 1)  # scratch WAW vs ACT g2
                        nc.vector.tensor_scalar(
                            out=scratch[:, SPLIT_ACT:],
                            in0=sneg[:, SPLIT_ACT:],
                            scalar1=top8[:, 8 * g + K : 8 * g + K + 1],
                            scalar2=0.0,
                            op0=mybir.AluOpType.add,
                            op1=mybir.AluOpType.max,
                        )
                        nc.vector.tensor_reduce(
                            out=acc_sb[:, G : G + 1],
                            in_=scratch[:, SPLIT_ACT:],
                            axis=mybir.AxisListType.X,
                            op=mybir.AluOpType.add,
                        ).then_inc(act, 1)

        @block.gpsimd
        def _(gpsimd):
            gpsimd.memset(warm[:], 0.0).then_inc(warm_s, 1)
            gpsimd.dma_start(out=so_sb[:], in_=syoff.ap()).then_inc(so_s, 16)
            gpsimd.wait_ge(so_s, 16)
            for g in range(G):
                gpsimd.indirect_dma_start(
                    out=sy_sb[:, g : g + 1],
                    out_offset=None,
                    in_=x_flat,
                    in_offset=bass.IndirectOffsetOnAxis(
                        ap=so_sb[:, g : g + 1], axis=0
                    ),
                ).then_inc(gat, 16)
            # (1 - s_y) in bf16 for the local rows (Pool, not DVE: the DVE
            # stream must not block on the gathers, which queue behind the
            # first bulk chunk loads)
            gpsimd.wait_ge(gat, 16 * G)
            nc.gpsimd.tensor_scalar(
                out=sneg_loc[:],
                in0=sy_sb[:],
                scalar1=-1.0,
                scalar2=1.0,
                op0=mybir.AluOpType.mult,
                op1=mybir.AluOpType.add,
            ).then_inc(bias_s, 1)
            gpsimd.wait_ge(bias_s, 1)
            for g in range(G):
                gpsimd.dma_start(
                    out=sloc_d[bass.ts(g, 128)], in_=sneg_loc[:, g : g + 1]
                ).then_inc(sld, 16)
            gpsimd.wait_ge(sld, 64)
            gpsimd.collective_compute(
                "AllGather",
                mybir.AluOpType.bypass,
                replica_groups=[list(range(NCORES))],
                ins=[sloc_d[:]],
                outs=[sfull_d[:]],
            ).then_inc(cc, 1)
            # (InstPartitionBroadcast would replicate on-chip and save the
            # ~2.9us of bus time this 1MB broadcast costs, but this
            # toolchain's codegen rejects it — keep the DMA broadcast.)
            gpsimd.wait_ge(cc, 1)
            gpsimd.dma_start(
                out=sneg[:], in_=sfull_d.ap().partition_broadcast(128)
            ).then_inc(tb_s, 16)

        @block.scalar
        def _(scalar):
            # warm the relu table while stage 1 runs
            scalar.wait_ge(warm_s, 1)
            nc.scalar.activation(
                out=warm[:],
                in_=warm[:],
                func=mybir.ActivationFunctionType.Relu,
            )
            scalar.wait_ge(tb_s, 16)
            for g in range(G):
                w = B if g < G - 1 else SPLIT_ACT
                scalar.wait_ge(tcp, G * (repeat - 1) + g + 1)
                nc.scalar.activation(
                    out=scratch[:, :w],
                    in_=sneg[:, :w],
                    func=mybir.ActivationFunctionType.Relu,
                    bias=tbias[:, g : g + 1],
                    scale=1.0,
                    accum_out=acc_sb[:, g : g + 1],
                ).then_inc(act, 1)

    return nc


def _get_nc(repeat: int = 1):
    key = ("nc", repeat)
    if key not in _NC_CACHE:
        _NC_CACHE[key] = _build_nc(repeat)
    return _NC_CACHE[key]


def _in_maps(x, y):
    x = np.ascontiguousarray(np.asarray(x, dtype=np.float32))
    y = np.asarray(y).astype(np.int64).reshape(B)
    assert x.shape == (B, V)
    in_maps = []
    r = np.arange(RPC, dtype=np.int64)
    for c in range(NCORES):
        rows = slice(c * RPC, (c + 1) * RPC)
        yl = y[rows]
        off = (r * V + yl).astype(np.int32).reshape(G, 128).T.copy()
        in_maps.append({"x": x[rows], "syoff": off})
    return in_maps


def _run(x, y, trace=False):
    nc = _get_nc()
    in_maps = _in_maps(x, y)
    res = run_bass_kernel_spmd(nc, in_maps, list(range(NCORES)), trace=trace)
    total = 0.0
    for c in range(NCORES):
        total += float(res.results[c]["partial"].astype(np.float64).sum())
    out = np.array(total / (float(B) * float(B)), dtype=np.float32)
    return out, res


def kernel(x, y, k):
    assert int(k) == K
    out, _ = _run(x, y, trace=False)
    return out


# revision 36
# speedup vs baseline: 1.0029x; 1.0029x over previous
"""Trainium2 Bass kernel for nn_Loss5 (topk_masking).

reference:
    s_topk = top_k(x, 6)[0][:, 5]            # 6th largest per row, [B]
    s_y    = x[arange(B), y]                 # label score, [B]
    out    = mean(relu(1 + s_topk[None,:] - s_y[:,None]))   # over [B,B]

Strategy (8 cores, data-parallel over rows):
  - each core gets 512 rows of x ([512, 50257] f32, contiguous slice)
  - stage 1: exact per-row top-8 via the DVE Max8 instruction
    (nc.vector.max) over column chunks, then a final Max8 over the
    per-chunk candidates; t_j = 6th value (local rows only).
  - the loss is sharded over j (columns of the [B_i, B_j] clamp
    matrix): core c computes P_c = sum_{j in rows_c} sum_i
    relu((1 - s_i) + t_j).  This needs ALL s_i but only LOCAL t_j, so
    the collective moves to the *front* where it is hidden behind the
    stage-1 DMA stream: gather s_y by indirect DMA (~us), compute
    (1 - s_y), AllGather it ([B] floats, tiny), broadcast to a
    [128, B] SBUF tile.  Per row-group, one ACT pass
    relu(sneg_i + t_j) with accum_out runs as soon as that group's
    top-k lands, overlapped with the remaining DMA.
  - the last group streams in ~2.4k-col chunks ending in a graduated
    tail (1700..400) so the DVE Max8 drain after the final byte is
    ~2.5us instead of 8.8us; the final loss pass is split between ACT
    (relu+accum fused) and DVE (4x-mode relu + reduce) so both legs
    finish together ~6.6us after the last byte.
  - host: sum 8x[128,G+1] partials, divide by B^2.

Raw bass blocks (not Tile): the toolchain's DMA pseudo-instructions
support only ONE attached sync wait, and Tile's wait emission is not
transitively minimal (slot reuse emits RAW + WAW waits on one DMA).
With explicit semaphores the WAW is implied transitively and every DMA
carries at most one wait.

Empirical notes (all verified on the 8-core axon setup):
  - cost-model sim 299.1us/core; measured stage-1 slope ~255us/iter
    (403GB/s/core effective, ~11% above the model's 360) -> ~268us.
  - DMA completion semaphores count to +16 PROGRESSIVELY: waiting >=1
    on a broadcast released readers onto a half-written tile (NaN).
  - an engine's then_inc can lead its SBUF write visibility for a
    cross-engine reader: ACT biasing directly off top8 gated by fmx
    read stale data (7.8e-2 err); the tensor_copy hop is REQUIRED.
  - tensor_scalar with accum_out: op1 becomes the REDUCTION op, so
    relu+sum needs two instructions (or ACT's fused activation).
  - InstPartitionBroadcast (would save the 2.9us broadcast DMA) is
    rejected by this toolchain's codegen (visitInstISA).
  - untested future path: dma prepare/trigger to pre-stage the final
    store's 1.27us config+DGE delay off the tail.
"""

import sys

import numpy as np

if "/opt/trn_rl_repo" not in sys.path:
    sys.path.insert(0, "/opt/trn_rl_repo")

import concourse.bass as bass
import concourse.mybir as mybir
from concourse.bass_utils import run_bass_kernel_spmd

B = 4096
V = 50257
NCORES = 8
RPC = B // NCORES          # rows per core = 512
G = RPC // 128             # row groups of 128 partitions = 4
K = 5                      # s_topk = (K+1)-th largest = top8[:, 5]

# column chunking for Max8 (input free size must be <= 16384).
# groups 0..G-2: 6 roughly equal chunks.  last group: small equal chunks
# (~2400: keeps the DVE caught up with the DMA stream, DVE(w)+sem <
# DMA(w)) ending in a graduated tail so the DVE drain after the last
# byte shrinks from 8.8us (one big chunk) to ~2us.
def _even_chunks(total, n):
    out, c0 = [], 0
    base, rem = total // n, total % n
    for j in range(n):
        w = base + (1 if j < rem else 0)
        out.append((c0, w))
        c0 += w
    return out

_CHUNKS_STD = _even_chunks(V, 6)            # widths 8377/8376, max 8377
# graduated drain tail: DVE backlog recursion
#   lag(c) = max(lag(c-1) - DMA(w_c), 0.9us) + DVE(w_c)
# converges to ~1.7us with this profile (vs 8.8us for a full-width
# last chunk), robust across the measured DMA-rate range.
_TAILS = [2100, 1800, 1500, 1250, 1050, 880, 740, 620, 520, 440]
_CHUNKS_LAST = _even_chunks(V - sum(_TAILS), 16)
for _w in _TAILS:
    _c0 = _CHUNKS_LAST[-1][0] + _CHUNKS_LAST[-1][1]
    _CHUNKS_LAST.append((_c0, _w))
assert _CHUNKS_LAST[-1][0] + _CHUNKS_LAST[-1][1] == V
_GROUP_CHUNKS = [_CHUNKS_STD] * (G - 1) + [_CHUNKS_LAST]
W0 = max(w for ch in _GROUP_CHUNKS for _, w in ch)   # slot width
NCHUNK_TOT = sum(len(ch) for ch in _GROUP_CHUNKS)    # 25
# per-group candidate offsets into cand (8 slots per chunk)
_CAND_OFF = []
_off = 0
for ch in _GROUP_CHUNKS:
    _CAND_OFF.append(_off)
    _off += 8 * len(ch)
CANDW = _off

NSLOT = 5                  # x-tile load slots

# the last group's loss pass is split between ACT (relu+accum fused,
# 0.833 ns/elem) and DVE (4x-mode relu 0.26 + 1x reduce 1.04 ns/elem)
# so both legs finish together on the tail.
SPLIT_ACT = 1525           # ACT columns; DVE gets B - SPLIT_ACT = 2571

_NC_CACHE = {}


def _build_nc(repeat: int = 1):
    f32 = mybir.dt.float32
    bf16 = mybir.dt.bfloat16
    i32 = mybir.dt.int32

    nc = bass.Bass()
    x = nc.declare_dram_parameter("x", [RPC, V], f32, isOutput=False)
    syoff = nc.declare_dram_parameter("syoff", [128, G], i32, isOutput=False)
    partial = nc.declare_dram_parameter("partial", [128, G + 1], f32, isOutput=True)

    sloc_d = nc.dram_tensor("sloc_cc_in", [RPC], bf16)
    sfull_d = nc.dram_tensor("sfull_cc_out", [B], bf16, addr_space="Shared")

    x_flat = x.ap().rearrange("a b -> (a b)")[:, None]

    from contextlib import ExitStack

    with ExitStack() as ctx:
        slots = ctx.enter_context(nc.sbuf_tensor("slots", [128, NSLOT * W0], f32))
        cand = ctx.enter_context(nc.sbuf_tensor("cand", [128, CANDW], f32))
        top8 = ctx.enter_context(nc.sbuf_tensor("top8", [128, G * 8], f32))
        tbias = ctx.enter_context(nc.sbuf_tensor("tbias", [128, G], f32))
        sy_sb = ctx.enter_context(nc.sbuf_tensor("sy", [128, G], f32))
        sneg_loc = ctx.enter_context(nc.sbuf_tensor("snegl", [128, G], bf16))
        acc_sb = ctx.enter_context(nc.sbuf_tensor("acc", [128, G + 1], f32))
        so_sb = ctx.enter_context(nc.sbuf_tensor("so", [128, G], i32))
        sneg = ctx.enter_context(nc.sbuf_tensor("sneg", [128, B], bf16))
        scratch = ctx.enter_context(nc.sbuf_tensor("scratch", [128, B], bf16))
        warm = ctx.enter_context(nc.sbuf_tensor("warm", [128, 8], f32))
        ld0 = ctx.enter_context(nc.semaphore("ld0"))
        ld1 = ctx.enter_context(nc.semaphore("ld1"))
        ld2 = ctx.enter_context(nc.semaphore("ld2"))
        ld3 = ctx.enter_context(nc.semaphore("ld3"))
        ld4 = ctx.enter_context(nc.semaphore("ld4"))
        mx = ctx.enter_context(nc.semaphore("mx"))
        so_s = ctx.enter_context(nc.semaphore("so_s"))
        gat = ctx.enter_context(nc.semaphore("gat"))
        bias_s = ctx.enter_context(nc.semaphore("bias_s"))
        tcp = ctx.enter_context(nc.semaphore("tcp"))
        sld = ctx.enter_context(nc.semaphore("sld"))
        cc = ctx.enter_context(nc.semaphore("cc"))
        tb_s = ctx.enter_context(nc.semaphore("tb_s"))
        act = ctx.enter_context(nc.semaphore("act"))
        outs = ctx.enter_context(nc.semaphore("outs"))
        fmx = ctx.enter_context(nc.semaphore("fmx"))
        warm_s = ctx.enter_context(nc.semaphore("warm_s"))
        block = ctx.enter_context(nc.Block())
        ld_sems = [ld0, ld1, ld2, ld3, ld4]

        @block.sync
        def _(sync):
            k = 0
            for rep in range(repeat):
                for g in range(G):
                    for j, (c0, w) in enumerate(_GROUP_CHUNKS[g]):
                        if k >= NSLOT:
                            # reader of this slot's previous contents done
                            # (transitively implies the previous load completed)
                            sync.wait_ge(mx, k - NSLOT + 1)
                        s = (k % NSLOT) * W0
                        cs = (c0 + rep * 1237) % (V - w) if rep else c0
                        sync.dma_start(
                            out=slots[:, s : s + w],
                            in_=x[g * 128 : (g + 1) * 128, cs : cs + w],
                        ).then_inc(ld_sems[k % NSLOT], 16)
                        k += 1
            sync.wait_ge(act, G + 1)
            sync.dma_start(out=partial.ap(), in_=acc_sb[:]).then_inc(outs, 16)
            sync.wait_ge(outs, 16)

        @block.vector
        def _(vector):
            k = 0
            for rep in range(repeat):
                for g in range(G):
                    chunks = _GROUP_CHUNKS[g]
                    co = _CAND_OFF[g]
                    for j, (c0, w) in enumerate(chunks):
                        s = (k % NSLOT) * W0
                        vector.wait_ge(ld_sems[k % NSLOT], 16 * (k // NSLOT + 1))
                        nc.vector.max(
                            cand[:, co + 8 * j : co + 8 * j + 8],
                            slots[:, s : s + w],
                        ).then_inc(mx, 1)
                        k += 1
                    vector.wait_ge(mx, k)
                    nc.vector.max(
                        top8[:, 8 * g : 8 * g + 8],
                        cand[:, co : co + 8 * len(chunks)],
                    ).then_inc(fmx, 1)
                    vector.wait_ge(fmx, rep * G + g + 1)
                    nc.vector.tensor_copy(
                        tbias[:, g : g + 1], top8[:, 8 * g + K : 8 * g + K + 1]
                    ).then_inc(tcp, 1)
                    if g == G - 1 and rep == repeat - 1:
                        # DVE leg of the last group's loss pass:
                        # relu(sneg + t) elementwise (4x mode on bf16), then
                        # a 1x reduce-add into acc[:, G].  ACT covers the
                        # first SPLIT_ACT columns concurrently.
                        vector.wait_ge(tb_s, 16)
                        vector.wait_ge(act, G - 1)  # scratch WAW vs ACT g2
                        nc.vector.tensor_scalar(
                            out=scratch[:, SPLIT_ACT:],
                            in0=sneg[:, SPLIT_ACT:],
                            scalar1=top8[:, 8 * g + K : 8 * g + K + 1],
                            scalar2=0.0,
                            op0=mybir.AluOpType.add,
                            op1=mybir.AluOpType.max,
                        )
                        # sum via tensor_scalar+accum_out (op1 is the
                        # REDUCTION op there): runs in 4x mode, unlike
                        # tensor_reduce which has no fast modes.  The
                        # elementwise identity out lands in sneg's upper
                        # columns (dead after this point).
                        nc.vector.tensor_scalar(
                            out=sneg[:, SPLIT_ACT:],
                            in0=scratch[:, SPLIT_ACT:],
                            scalar1=0.0,
                            scalar2=None,
                            op0=mybir.AluOpType.add,
                            op1=mybir.AluOpType.add,
                            accum_out=acc_sb[:, G : G + 1],
                        ).then_inc(act, 1)

        @block.gpsimd
        def _(gpsimd):
            gpsimd.memset(warm[:], 0.0).then_inc(warm_s, 1)
            gpsimd.dma_start(out=so_sb[:], in_=syoff.ap()).then_inc(so_s, 16)
            gpsimd.wait_ge(so_s, 16)
            for g in range(G):
                gpsimd.indirect_dma_start(
                    out=sy_sb[:, g : g + 1],
                    out_offset=None,
                    in_=x_flat,
                    in_offset=bass.IndirectOffsetOnAxis(
                        ap=so_sb[:, g : g + 1], axis=0
                    ),
                ).then_inc(gat, 16)
            # (1 - s_y) in bf16 for the local rows (Pool, not DVE: the DVE
            # stream must not block on the gathers, which queue behind the
            # first bulk chunk loads)
            gpsimd.wait_ge(gat, 16 * G)
            nc.gpsimd.tensor_scalar(
                out=sneg_loc[:],
                in0=sy_sb[:],
                scalar1=-1.0,
                scalar2=1.0,
                op0=mybir.AluOpType.mult,
                op1=mybir.AluOpType.add,
            ).then_inc(bias_s, 1)
            gpsimd.wait_ge(bias_s, 1)
            for g in range(G):
                gpsimd.dma_start(
                    out=sloc_d[bass.ts(g, 128)], in_=sneg_loc[:, g : g + 1]
                ).then_inc(sld, 16)
            gpsimd.wait_ge(sld, 64)
            gpsimd.collective_compute(
                "AllGather",
                mybir.AluOpType.bypass,
                replica_groups=[list(range(NCORES))],
                ins=[sloc_d[:]],
                outs=[sfull_d[:]],
            ).then_inc(cc, 1)
            # (InstPartitionBroadcast would replicate on-chip and save the
            # ~2.9us of bus time this 1MB broadcast costs, but this
            # toolchain's codegen rejects it — keep the DMA broadcast.)
            gpsimd.wait_ge(cc, 1)
            gpsimd.dma_start(
                out=sneg[:], in_=sfull_d.ap().partition_broadcast(128)
            ).then_inc(tb_s, 16)

        @block.scalar
        def _(scalar):
            # warm the relu table while stage 1 runs
            scalar.wait_ge(warm_s, 1)
            nc.scalar.activation(
                out=warm[:],
                in_=warm[:],
                func=mybir.ActivationFunctionType.Relu,
            )
            scalar.wait_ge(tb_s, 16)
            for g in range(G):
                w = B if g < G - 1 else SPLIT_ACT
                scalar.wait_ge(tcp, G * (repeat - 1) + g + 1)
                nc.scalar.activation(
                    out=scratch[:, :w],
                    in_=sneg[:, :w],
                    func=mybir.ActivationFunctionType.Relu,
                    bias=tbias[:, g : g + 1],
                    scale=1.0,
                    accum_out=acc_sb[:, g : g + 1],
                ).then_inc(act, 1)

    return nc


def _get_nc(repeat: int = 1):
    key = ("nc", repeat)
    if key not in _NC_CACHE:
        _NC_CACHE[key] = _build_nc(repeat)
    return _NC_CACHE[key]


def _in_maps(x, y):
    x = np.ascontiguousarray(np.asarray(x, dtype=np.float32))
    y = np.asarray(y).astype(np.int64).reshape(B)
    assert x.shape == (B, V)
    in_maps = []
    r = np.arange(RPC, dtype=np.int64)
    for c in range(NCORES):
        rows = slice(c * RPC, (c + 1) * RPC)
        yl = y[rows]
        off = (r * V + yl).astype(np.int32).reshape(G, 128).T.copy()
        in_maps.append({"x": x[rows], "syoff": off})
    return in_maps


def _run(x, y, trace=False):
    nc = _get_nc()
    in_maps = _in_maps(x, y)
    res = run_bass_kernel_spmd(nc, in_maps, list(range(NCORES)), trace=trace)
    total = 0.0
    for c in range(NCORES):
        total += float(res.results[c]["partial"].astype(np.float64).sum())
    out = np.array(total / (float(B) * float(B)), dtype=np.float32)
    return out, res


def kernel(x, y, k):
    assert int(k) == K
    out, _ = _run(x, y, trace=False)
    return out
